# revision 18
# baseline (speedup 1.0000x reference)
"""ListNet loss Trainium2 kernel.

kernel(y_pred_scores [2048, 8192] f32, y_true_seqs [2048, 512] int) -> () f32

Strategy: pure data parallel over the batch dim across 8 NeuronCores
(256 rows/core, 2 tiles of 128 rows). The score gather
g[r, l] = scores[r, idx[r, l]] is INVERTED into GPSIMD local_scatter
passes (local_scatter supports fully independent per-partition index
maps, unlike ap_gather's 16-partition-shared lists):

  - host builds inv[r, n] = first (reversed-order) sequence position l
    with idx[r, l] == n, else -1; local_scatter scans the fp16 score
    row and writes each used column's value to its first sequence
    position. Cost is ~2.9ns per scanned column on the Pool engine, all
    128 partitions productive.
  - 256 extra data columns hold PADV=-1024 and scatter onto the pad
    positions (pads sit at the reversed-sequence head, [0, padcount)),
    so exp gives exact 0 there with no pad masking on device.
  - duplicated indices (~16/row) are patched by one tiny fix-up scatter
    (fix1[r, l1] = l2) sourcing the value already placed at the first
    occurrence; pass 1 leaves exact zeros at l2 so add-merging is
    exact. Multiplicity >= 3 stays unpatched (~1e-4 relative effect on
    the final scalar, the gate is 2e-2).
  - scores travel as fp16 (local_scatter moves 2-byte data).

Schedule notes (the Pool engine is the bottleneck and DVE stalls while
a local_scatter runs, so DVE work is packed into forced Pool gaps):
  - tile 0's columns run as three pieces (2112/3168/3168) in separate
    dsts so the first scatter starts as soon as ~1MB landed; DVE merges
    the dsts in the Pool gap before the fix-up.
  - DMA waves are staggered by tiny dependent SBUF->SBUF copies into
    the next wave's destination head (real WAW deps -- the DMA engines
    fair-share among active transfers, and the list scheduler reorders
    anything without a data dependency). All DMA configs live on the
    Sync engine, which runs nothing else, so no compute op can
    head-block them.
  - tile 1's scatter reads 2 junk data columns written by a DVE copy of
    g_0, forcing the Pool gap in which tile 0's merges and fix-up run.

Per tile the device computes e = exp(g), S = prefix sum of e (sequence
pre-reversed on host -> suffix logsumexp denominator), and accumulates
sumG = sum(g) (by the merge op) and sumLn = sum(ln(S + eps)) (by the Ln
activation). A [P, 1] column C = ln(0 + eps) captures the HW Ln value
at pad positions so the host cancels pads exactly:
  row_ll = sumG - sumLn - padcount * (PADV - C);  used = padcount < L
  result = -sum(row_ll over used) / sum(used).
"""

import numpy as np

B, N, L = 2048, 8192, 512
NCORES = 8
BL = B // NCORES  # 256 rows per core
P = 128
NT = BL // P  # = 2 tiles of 128 rows per core
PADC = 256  # extra data columns used to scatter PADV onto pad positions
PADV = -1024.0  # exact in fp16; exp() underflows to exact 0
EPS = 2.0**-126
# tile-0 column pieces (of N + PADC total); tile 1 runs one full scatter
PIECES = [(0, 2112), (2112, 3168), (5280, 3168)]

TRACE = False
LAST_RESULTS = None

_cache = {}


def _build():
    import concourse.bacc as bacc
    import concourse.mybir as mybir
    import concourse.tile as tile

    f32 = mybir.dt.float32
    f16 = mybir.dt.float16
    i16 = mybir.dt.int16
    Alu = mybir.AluOpType
    Act = mybir.ActivationFunctionType

    nc = bacc.Bacc("TRN2", target_bir_lowering=False, debug=False)
    scores = nc.dram_tensor("scores", [BL, N], f16, kind="ExternalInput").ap()
    inv = nc.dram_tensor("inv", [BL, N + PADC], i16, kind="ExternalInput").ap()
    fix1 = nc.dram_tensor("fix1", [BL, L], i16, kind="ExternalInput").ap()
    # out columns: [sumG(t0), sumLn(t0), sumG(t1), sumLn(t1), C]
    out = nc.dram_tensor("out", [P, 2 * NT + 1], f32, kind="ExternalOutput").ap()

    r0 = slice(0, P)
    r1 = slice(P, 2 * P)

    with tile.TileContext(nc) as tc:
        with (
            tc.tile_pool(name="const", bufs=1) as cpool,
            tc.tile_pool(name="big", bufs=1) as bpool,
            tc.tile_pool(name="work", bufs=2) as pool,
        ):
            epsb = cpool.tile([P, 1], f32)
            nc.vector.memset(epsb[:], EPS)
            z = cpool.tile([P, 1], f32)
            nc.vector.memset(z[:], 0.0)
            stats = cpool.tile([P, 2 * NT + 1], f32)
            nc.scalar.activation(
                out=stats[:, 2 * NT :], in_=z[:], func=Act.Ln, bias=epsb[:],
                scale=1.0,
            )

            # ---- tile-0 pieces: staggered DMA waves + scatters
            f1_t = []
            for t, rr in enumerate((r0, r1)):
                f1 = pool.tile([P, L], i16, tag="f1")
                nc.scalar.dma_start(out=f1[:], in_=fix1[rr, :])
                f1_t.append(f1)

            scp, ivp = [], []
            for k, (c0, w) in enumerate(PIECES):
                sc = bpool.tile([P, w], f16, tag=f"sc{k}")
                iv = bpool.tile([P, w], i16, tag=f"iv{k}")
                if k > 0:
                    # gate this wave's DMAs on the previous wave landing
                    nc.sync.dma_start(out=sc[:, 0:2], in_=scp[-1][:, 0:2])
                    nc.sync.dma_start(out=iv[:, 0:2], in_=ivp[-1][:, 0:2])
                ns = min(c0 + w, N) - c0  # score columns (rest are pads)
                nc.sync.dma_start(out=sc[:, :ns], in_=scores[r0, c0 : c0 + ns])
                if ns < w:
                    nc.vector.memset(sc[:, ns:], PADV)
                nc.sync.dma_start(out=iv[:], in_=inv[r0, c0 : c0 + w])
                scp.append(sc)
                ivp.append(iv)
            # ---- wave 4: tile 1 full width, gated on wave 2 so it overlaps
            # wave 3's transfer and the piece scatters
            sc1 = bpool.tile([P, N + PADC + 2], f16, tag="scT1")
            iv1 = bpool.tile([P, N + PADC + 2], i16, tag="ivT1")
            nc.scalar.dma_start(out=sc1[:, 0:2], in_=scp[-1][:, 2:4])
            nc.scalar.dma_start(out=iv1[:, 0:2], in_=ivp[-1][:, 2:4])
            nc.scalar.dma_start(out=sc1[:, :N], in_=scores[r1, :])
            nc.scalar.dma_start(out=iv1[:, : N + PADC], in_=inv[r1, :])
            nc.vector.memset(sc1[:, N : N + PADC], PADV)
            nc.vector.memset(iv1[:, N + PADC :], -1)

            gp = []
            for k, (c0, w) in enumerate(PIECES):
                g = pool.tile([P, L], f16, tag=f"gp{k}")
                nc.gpsimd.local_scatter(
                    out_ap=g[:], data_ap=scp[k][:], idxs_ap=ivp[k][:],
                    channels=P, num_elems=L, num_idxs=w,
                )
                gp.append(g)
            with tc.high_priority():
                m12 = pool.tile([P, L], f16, tag="m12")
                nc.vector.tensor_tensor(
                    out=m12[:], in0=gp[0][:], in1=gp[1][:], op=Alu.add
                )
                g1_0 = pool.tile([P, L], f16, tag="g1")
                nc.vector.tensor_tensor(
                    out=g1_0[:], in0=m12[:], in1=gp[2][:], op=Alu.add
                )
                gf_0 = pool.tile([P, L], f16, tag="gf")
                nc.gpsimd.local_scatter(
                    out_ap=gf_0[:], data_ap=g1_0[:], idxs_ap=f1_t[0][:],
                    channels=P, num_elems=L, num_idxs=L,
                )
                g_0 = pool.tile([P, L], f32, tag="g")
                nc.vector.scalar_tensor_tensor(
                    out=g_0[:],
                    in0=g1_0[:],
                    scalar=1.0,
                    in1=gf_0[:],
                    op0=Alu.mult,
                    op1=Alu.add,
                    accum_out=stats[:, 0:1],
                )
                # junk data columns gate tile 1's scatter behind g_0's merge
                nc.vector.tensor_copy(
                    out=sc1[:, N + PADC :], in_=g_0[:, 0:2]
                )
            e_0 = pool.tile([P, L], f32, tag="e")
            nc.scalar.activation(out=e_0[:], in_=g_0[:], func=Act.Exp)

            # ---- tile 1: full-width scatter -> fix-up
            g1_1 = pool.tile([P, L], f16, tag="g1")
            nc.gpsimd.local_scatter(
                out_ap=g1_1[:], data_ap=sc1[:], idxs_ap=iv1[:],
                channels=P, num_elems=L, num_idxs=N + PADC + 2,
            )
            with tc.high_priority():
                gf_1 = pool.tile([P, L], f16, tag="gf")
                nc.gpsimd.local_scatter(
                    out_ap=gf_1[:], data_ap=g1_1[:], idxs_ap=f1_t[1][:],
                    channels=P, num_elems=L, num_idxs=L,
                )
            g_1 = pool.tile([P, L], f32, tag="g")
            nc.vector.scalar_tensor_tensor(
                out=g_1[:],
                in0=g1_1[:],
                scalar=1.0,
                in1=gf_1[:],
                op0=Alu.mult,
                op1=Alu.add,
                accum_out=stats[:, 2:3],
            )
            e_1 = pool.tile([P, L], f32, tag="e")
            nc.scalar.activation(out=e_1[:], in_=g_1[:], func=Act.Exp)

            # ---- chains: scan + ln(+accum)
            for t, e in enumerate((e_0, e_1)):
                S = pool.tile([P, L], f32, tag="s")
                nc.vector.tensor_tensor_scan(
                    out=S[:], data0=e[:], data1=e[:], initial=0.0,
                    op0=Alu.add, op1=Alu.bypass,
                )
                lnS = pool.tile([P, L], f32, tag="lns")
                nc.scalar.activation(
                    out=lnS[:], in_=S[:], func=Act.Ln, bias=epsb[:], scale=1.0,
                    accum_out=stats[:, 2 * t + 1 : 2 * t + 2],
                )
            nc.sync.dma_start(out=out[:], in_=stats[:])

    nc.compile()
    return nc


def _get_nc():
    if "nc" not in _cache:
        _cache["nc"] = _build()
    return _cache["nc"]


def _host_prep(y_pred_scores, y_true_seqs):
    scores16 = np.ascontiguousarray(y_pred_scores.astype(np.float16))
    seqs = y_true_seqs.astype(np.int16)
    # reversed along L so the on-device forward scan is the suffix sum
    seqs_rev = np.ascontiguousarray(seqs[:, ::-1])

    valid = seqs_rev != -1
    padcount = (~valid).sum(axis=1).astype(np.int64)  # pads at [0, padcount)
    idx = np.clip(seqs_rev.astype(np.int64), 0, None)

    # occurrence ranks: for each (row, column) group, rank positions by l
    r = np.repeat(np.arange(B, dtype=np.int64)[:, None], L, axis=1)
    ll = np.tile(np.arange(L, dtype=np.int64)[None, :], (B, 1))
    rf, lf, if_ = r[valid], ll[valid], idx[valid]
    key = rf * N + if_
    order = np.lexsort((lf, key))
    ks, ls = key[order], lf[order]
    first = np.ones(ks.size, dtype=bool)
    first[1:] = ks[1:] != ks[:-1]
    gstart = np.maximum.accumulate(np.where(first, np.arange(ks.size), 0))
    rank = np.arange(ks.size) - gstart
    l0 = ls[gstart]  # first-occurrence position of each entry's group
    rows_s = rf[order]

    inv = np.full((B, N + PADC), -1, dtype=np.int16)
    m0 = rank == 0
    inv[rows_s[m0], if_[order][m0]] = ls[m0].astype(np.int16)
    # pad columns: column N+j scatters PADV onto pad position j
    j = np.arange(PADC, dtype=np.int16)[None, :]
    inv[:, N:] = np.where(j < padcount[:, None], j, -1)

    fix1 = np.full((B, L), -1, dtype=np.int16)
    m1 = rank == 1
    fix1[rows_s[m1], l0[m1]] = ls[m1].astype(np.int16)

    return scores16, inv, fix1, padcount


def kernel(y_pred_scores: np.ndarray, y_true_seqs: np.ndarray) -> np.ndarray:
    global LAST_RESULTS
    from concourse.bass_utils import run_bass_kernel_spmd

    nc = _get_nc()
    scores16, inv, fix1, padcount = _host_prep(y_pred_scores, y_true_seqs)

    in_maps = []
    for c in range(NCORES):
        sl = slice(c * BL, (c + 1) * BL)
        in_maps.append(
            {
                "scores": scores16[sl],
                "inv": inv[sl],
                "fix1": fix1[sl],
            }
        )

    res = run_bass_kernel_spmd(nc, in_maps, list(range(NCORES)), trace=TRACE)
    LAST_RESULTS = res

    padv = float(np.float32(np.float16(PADV)))
    total_ll = 0.0
    n_used = 0.0
    for c in range(NCORES):
        st = res.results[c]["out"].astype(np.float64)  # [P, 2*NT+1]
        C = st[:, 2 * NT]
        for t in range(NT):
            rows = slice(c * BL + t * P, c * BL + (t + 1) * P)
            pc = padcount[rows]
            sumd = st[:, 2 * t] - st[:, 2 * t + 1]
            # pads contributed (PADV - C) each; remove them
            row_ll = sumd - pc * (padv - C)
            used = pc < L
            total_ll += np.where(used, row_ll, 0.0).sum()
            n_used += used.sum()

    if n_used > 0:
        return np.float32(-total_ll / n_used)
    return np.float32(0.0)
